# revision 3
# baseline (speedup 1.0000x reference)
"""DWAC (deep weighted-averaging classifier) kernel for 8x Trainium2 NeuronCores.

Problem: 3-layer MLP -> z [8192, 10], pairwise Gaussian kernel over all rows,
per-class kernel-weight aggregation, log-probs + NLL loss.

Strategy (data-parallel over rows, per the sharding hint):
 - Host: stable-sort rows by class label (so the class aggregation becomes
   contiguous segment sums along the j axis), shard rows 1024/core, pre-transpose
   x slabs to [xdim, rows] and cast matmul operands to bf16.
 - Device (identical SPMD program on 8 cores):
     * MLP computed in transposed layout (hT = W.T @ xT) so only weights (already
       [K, M]) are stationary and no on-device transposes are needed.
     * z is augmented to 12 dims: moving j-vector [z, |z|^2, 1], stationary
       i-vector [-2z, 1, |z|^2]; their dot product is the squared distance, so
       one K=12 matmul emits distance tiles straight into PSUM.
     * One AllGather (bf16 [12, 1024] per core) replicates the moving vectors.
     * ScalarE: exp(-0.5*gamma*d) PSUM->SBUF in [128, 2048] tiles.
     * VectorE: per-class segment row-sums (class boundaries are compile-time
       constants derived from y), then eps/diagonal fixup, log-probs, NLL.
 - Host: concat per-core prob slabs, invert the row permutation, sum loss parts.
"""
import sys

sys.path.insert(0, "/opt/trn_rl_repo")

import numpy as np
import ml_dtypes

import jax

jax.config.update("jax_compilation_cache_dir", "/tmp/jaxcache")
jax.config.update("jax_persistent_cache_min_compile_time_secs", 0.0)

import concourse.bass as bass
import concourse.bacc as bacc
import concourse.tile as tile
import concourse.mybir as mybir
from concourse.bass_utils import run_bass_kernel_spmd

dt = mybir.dt
AF = mybir.ActivationFunctionType
ALU = mybir.AluOpType
AX = mybir.AxisListType
BF16 = ml_dtypes.bfloat16

N = 8192
NC = 8
SLAB = N // NC          # 1024 rows per core
XD, D1, D2, ZD = 1024, 512, 256, 10
NCLS = 10
KD = 12                 # augmented z dim
GAMMA = 1.0
EPS = 1e-6
IB = SLAB // 128        # 8 i-blocks of 128 rows per core
JCH = 512               # moving free-dim per matmul
NJC = N // JCH          # 16 j-chunks
PSG = 4                 # j-chunks per PSUM group (4 banks = [128, 2048])

_compiled = {}


def _build(offs):
    """Build + compile the SPMD device program. offs: class segment offsets."""
    key = tuple(offs)
    if key in _compiled:
        return _compiled[key]

    nc = bacc.Bacc("TRN2", target_bir_lowering=False, debug=False,
                   enable_asserts=True, num_devices=NC)

    xT = nc.dram_tensor("xT", [XD, SLAB], dt.bfloat16, kind="ExternalInput")
    w1 = nc.dram_tensor("w1", [XD, D1], dt.bfloat16, kind="ExternalInput")
    w2 = nc.dram_tensor("w2", [D1, D2], dt.bfloat16, kind="ExternalInput")
    w3 = nc.dram_tensor("w3", [D2, ZD], dt.bfloat16, kind="ExternalInput")
    b1 = nc.dram_tensor("b1", [D1], dt.float32, kind="ExternalInput")
    b2 = nc.dram_tensor("b2", [D2], dt.float32, kind="ExternalInput")
    b3 = nc.dram_tensor("b3", [ZD], dt.float32, kind="ExternalInput")
    onehot = nc.dram_tensor("onehot", [SLAB, NCLS], dt.float32, kind="ExternalInput")

    probs_o = nc.dram_tensor("probs", [SLAB, NCLS], dt.float32, kind="ExternalOutput")
    loss_o = nc.dram_tensor("loss", [1, 1], dt.float32, kind="ExternalOutput")

    # collective bounce buffers
    zmov_d = nc.dram_tensor("zmov_d", [KD, SLAB], dt.bfloat16)
    zall_d = nc.dram_tensor("zall_d", [NC * KD, SLAB], dt.bfloat16, addr_space="Shared")

    with tile.TileContext(nc) as tc:
        with tc.tile_pool(name="per", bufs=1) as per:
            # ---- persistent SBUF tiles ----
            xts = [per.tile([128, SLAB], dt.bfloat16, tag=f"xt{k}", name=f"xt{k}")
                   for k in range(XD // 128)]
            w1s = [per.tile([128, D1], dt.bfloat16, tag=f"w1_{k}", name=f"w1_{k}")
                   for k in range(XD // 128)]
            w2s = [per.tile([128, D2], dt.bfloat16, tag=f"w2_{k}", name=f"w2_{k}")
                   for k in range(D1 // 128)]
            w3s = [per.tile([128, ZD], dt.bfloat16, tag=f"w3_{k}", name=f"w3_{k}")
                   for k in range(D2 // 128)]
            h1s = [per.tile([128, SLAB], dt.bfloat16, tag=f"h1_{k}", name=f"h1_{k}")
                   for k in range(D1 // 128)]
            h2s = [per.tile([128, SLAB], dt.bfloat16, tag=f"h2_{k}", name=f"h2_{k}")
                   for k in range(D2 // 128)]
            b1s = [per.tile([128, 1], dt.float32, tag=f"b1_{k}", name=f"b1_{k}")
                   for k in range(D1 // 128)]
            b2s = [per.tile([128, 1], dt.float32, tag=f"b2_{k}", name=f"b2_{k}")
                   for k in range(D2 // 128)]
            b3s = per.tile([ZD, 1], dt.float32, tag="b3s", name="b3s")
            zT = per.tile([ZD, SLAB], dt.float32, tag="zT", name="zT")
            zsq = per.tile([ZD, SLAB], dt.float32, tag="zsq", name="zsq")
            n_sb = per.tile([1, SLAB], dt.float32, tag="n_sb", name="n_sb")
            zmov_bf = per.tile([ZD, SLAB], dt.bfloat16, tag="zmov_bf", name="zmov_bf")
            n_bf = per.tile([1, SLAB], dt.bfloat16, tag="n_bf", name="n_bf")
            one_bf = per.tile([1, SLAB], dt.bfloat16, tag="one_bf", name="one_bf")
            zstat = per.tile([KD, SLAB], dt.bfloat16, tag="zstat", name="zstat")
            ones10 = per.tile([ZD, 1], dt.float32, tag="ones10", name="ones10")
            ones128 = per.tile([128, 1], dt.float32, tag="ones128", name="ones128")
            loss_parts = per.tile([128, IB], dt.float32, tag="loss_parts",
                                  name="loss_parts")
            zgs = [per.tile([KD, SLAB], dt.bfloat16, tag=f"zg{r}", name=f"zg{r}")
                   for r in range(NC)]

            # ---- input DMAs ----
            for k in range(XD // 128):
                nc.sync.dma_start(xts[k][:], xT[k * 128:(k + 1) * 128, :])
                nc.sync.dma_start(w1s[k][:], w1[k * 128:(k + 1) * 128, :])
            for k in range(D1 // 128):
                nc.sync.dma_start(w2s[k][:], w2[k * 128:(k + 1) * 128, :])
                nc.sync.dma_start(b1s[k][:], b1[k * 128:(k + 1) * 128][:, None])
            for k in range(D2 // 128):
                nc.sync.dma_start(w3s[k][:], w3[k * 128:(k + 1) * 128, :])
                nc.sync.dma_start(b2s[k][:], b2[k * 128:(k + 1) * 128][:, None])
            nc.sync.dma_start(b3s[:], b3[:][:, None])
            nc.vector.memset(ones10[:], 1.0)
            nc.vector.memset(ones128[:], 1.0)

            # ---- phase 1: MLP (transposed activations) ----
            with tc.tile_pool(name="mlpp", bufs=2, space="PSUM") as mlpp, \
                 tc.tile_pool(name="zp", bufs=2, space="PSUM") as zp, \
                 tc.tile_pool(name="np_", bufs=1, space="PSUM") as np_:
                for d1b in range(D1 // 128):
                    for ic in range(SLAB // JCH):
                        pt = mlpp.tile([128, JCH], dt.float32, tag="mlp_ps")
                        for kk in range(XD // 128):
                            nc.tensor.matmul(
                                pt[:],
                                w1s[kk][:, d1b * 128:(d1b + 1) * 128],
                                xts[kk][:, ic * JCH:(ic + 1) * JCH],
                                start=(kk == 0), stop=(kk == XD // 128 - 1))
                        nc.scalar.activation(
                            h1s[d1b][:, ic * JCH:(ic + 1) * JCH], pt[:],
                            AF.Relu, bias=b1s[d1b][:], scale=1.0)
                for d2b in range(D2 // 128):
                    for ic in range(SLAB // JCH):
                        pt = mlpp.tile([128, JCH], dt.float32, tag="mlp_ps")
                        for kk in range(D1 // 128):
                            nc.tensor.matmul(
                                pt[:],
                                w2s[kk][:, d2b * 128:(d2b + 1) * 128],
                                h1s[kk][:, ic * JCH:(ic + 1) * JCH],
                                start=(kk == 0), stop=(kk == D1 // 128 - 1))
                        nc.scalar.activation(
                            h2s[d2b][:, ic * JCH:(ic + 1) * JCH], pt[:],
                            AF.Identity, bias=b2s[d2b][:], scale=1.0)
                for ic in range(SLAB // JCH):
                    zt_ps = zp.tile([ZD, JCH], dt.float32, tag="zt_ps")
                    for kk in range(D2 // 128):
                        nc.tensor.matmul(
                            zt_ps[:], w3s[kk][:],
                            h2s[kk][:, ic * JCH:(ic + 1) * JCH],
                            start=(kk == 0), stop=(kk == D2 // 128 - 1))
                    nc.scalar.activation(
                        zT[:, ic * JCH:(ic + 1) * JCH], zt_ps[:],
                        AF.Identity, bias=b3s[:], scale=1.0)
                # squared norms: n = ones10.T @ (z*z)
                nc.vector.tensor_mul(zsq[:], zT[:], zT[:])
                n_ps = np_.tile([1, SLAB], dt.float32, tag="n_ps")
                for ic in range(SLAB // JCH):
                    nc.tensor.matmul(
                        n_ps[:, ic * JCH:(ic + 1) * JCH], ones10[:],
                        zsq[:, ic * JCH:(ic + 1) * JCH], start=True, stop=True)
                nc.scalar.copy(n_sb[:], n_ps[:])

            # ---- phase 1.5: build augmented z, AllGather ----
            # compute-engine partition base must be 32-aligned, so the
            # [12, *] stacks are assembled with DMA row writes instead.
            nc.vector.tensor_copy(zmov_bf[:], zT[:])
            nc.vector.tensor_copy(n_bf[:], n_sb[:])
            nc.vector.memset(one_bf[:], 1.0)
            nc.vector.tensor_scalar_mul(zstat[0:ZD, :], zT[:], -2.0)
            nc.sync.dma_start(zstat[ZD:ZD + 1, :], one_bf[:])
            nc.sync.dma_start(zstat[ZD + 1:KD, :], n_bf[:])

            nc.sync.dma_start(zmov_d[0:ZD, :], zmov_bf[:])
            nc.sync.dma_start(zmov_d[ZD:ZD + 1, :], n_bf[:])
            nc.sync.dma_start(zmov_d[ZD + 1:KD, :], one_bf[:])
            nc.gpsimd.collective_compute(
                "AllGather", ALU.bypass,
                replica_groups=[list(range(NC))],
                ins=[zmov_d[:]], outs=[zall_d[:]])
            for r in range(NC):
                nc.sync.dma_start(zgs[r][:], zall_d[r * KD:(r + 1) * KD, :])

            # ---- phase 2: pairwise distances -> exp -> class segment sums ----
            with tc.tile_pool(name="kpool", bufs=2) as kpool, \
                 tc.tile_pool(name="pairp", bufs=2, space="PSUM") as pairp, \
                 tc.tile_pool(name="small", bufs=3) as small:
                for ib in range(IB):
                    kbuf = kpool.tile([128, N], dt.float32, tag="kbuf")
                    st = zstat[:, ib * 128:(ib + 1) * 128]
                    for g in range(NJC // PSG):
                        pt = pairp.tile([128, PSG * JCH], dt.float32, tag="pair_ps")
                        for jj in range(PSG):
                            jc = g * PSG + jj
                            r, h = divmod(jc, SLAB // JCH)
                            nc.tensor.matmul(
                                pt[:, jj * JCH:(jj + 1) * JCH], st,
                                zgs[r][:, h * JCH:(h + 1) * JCH],
                                start=True, stop=True)
                        nc.scalar.activation(
                            kbuf[:, g * PSG * JCH:(g + 1) * PSG * JCH], pt[:],
                            AF.Exp, scale=-0.5 * GAMMA)
                    cd = small.tile([128, NCLS], dt.float32, tag="cd")
                    for c in range(NCLS):
                        lo, hi = offs[c], offs[c + 1]
                        if lo == hi:
                            nc.vector.memset(cd[:, c:c + 1], 0.0)
                        else:
                            nc.vector.reduce_sum(cd[:, c:c + 1], kbuf[:, lo:hi],
                                                 axis=AX.X)
                    oh = small.tile([128, NCLS], dt.float32, tag="oh")
                    nc.sync.dma_start(oh[:], onehot[ib * 128:(ib + 1) * 128, :])
                    cdf = small.tile([128, NCLS], dt.float32, tag="cdf")
                    # cdf = (cd + EPS) - onehot   (diagonal k_ii == 1 removal)
                    nc.vector.scalar_tensor_tensor(
                        cdf[:], cd[:], float(EPS), oh[:],
                        op0=ALU.add, op1=ALU.subtract)
                    rs = small.tile([128, 1], dt.float32, tag="rs")
                    nc.vector.reduce_sum(rs[:], cdf[:], axis=AX.X)
                    lcd = small.tile([128, NCLS], dt.float32, tag="lcd")
                    nc.scalar.activation(lcd[:], cdf[:], AF.Ln)
                    lrs = small.tile([128, 1], dt.float32, tag="lrs")
                    nc.scalar.activation(lrs[:], rs[:], AF.Ln)
                    pr = small.tile([128, NCLS], dt.float32, tag="pr")
                    nc.vector.tensor_scalar_sub(pr[:], lcd[:], lrs[:])
                    nc.sync.dma_start(probs_o[ib * 128:(ib + 1) * 128, :], pr[:])
                    tmp = small.tile([128, NCLS], dt.float32, tag="tmp")
                    nc.vector.tensor_mul(tmp[:], pr[:], oh[:])
                    nc.vector.reduce_sum(loss_parts[:, ib:ib + 1], tmp[:], axis=AX.X)

            # ---- phase 3: loss ----
            with tc.tile_pool(name="lossp", bufs=1, space="PSUM") as lossp:
                l_ps = lossp.tile([1, IB], dt.float32, tag="l_ps")
                nc.tensor.matmul(l_ps[:], ones128[:], loss_parts[:],
                                 start=True, stop=True)
                lsum = per.tile([1, 1], dt.float32, tag="lsum", name="lsum")
                nc.vector.reduce_sum(lsum[:], l_ps[:], axis=AX.X)
                lneg = per.tile([1, 1], dt.float32, tag="lneg", name="lneg")
                nc.vector.tensor_scalar_mul(lneg[:], lsum[:], -1.0)
                nc.sync.dma_start(loss_o[:], lneg[:])

    nc.compile()
    _compiled[key] = nc
    return nc


def _run(inputs, trace=False):
    x = np.asarray(inputs["x"], dtype=np.float32)
    y = np.asarray(inputs["y"])
    W1 = np.asarray(inputs["W1"], dtype=np.float32)
    b1 = np.asarray(inputs["b1"], dtype=np.float32)
    W2 = np.asarray(inputs["W2"], dtype=np.float32)
    b2 = np.asarray(inputs["b2"], dtype=np.float32)
    W3 = np.asarray(inputs["W3"], dtype=np.float32)
    b3 = np.asarray(inputs["b3"], dtype=np.float32)

    perm = np.argsort(y, kind="stable")
    yp = y[perm]
    counts = np.bincount(yp.astype(np.int64), minlength=NCLS)
    offs = [0]
    for c in range(NCLS):
        offs.append(offs[-1] + int(counts[c]))
    onehot = np.eye(NCLS, dtype=np.float32)[yp.astype(np.int64)]

    nc = _build(offs)

    w1b = W1.astype(BF16)
    w2b = W2.astype(BF16)
    w3b = W3.astype(BF16)
    in_maps = []
    for c in range(NC):
        rows = perm[c * SLAB:(c + 1) * SLAB]
        xTc = np.ascontiguousarray(x[rows].T).astype(BF16)
        in_maps.append({
            "xT": xTc, "w1": w1b, "w2": w2b, "w3": w3b,
            "b1": b1, "b2": b2, "b3": b3,
            "onehot": np.ascontiguousarray(onehot[c * SLAB:(c + 1) * SLAB]),
        })

    res = run_bass_kernel_spmd(nc, in_maps, list(range(NC)), trace=trace)

    probs_p = np.concatenate([res.results[c]["probs"] for c in range(NC)], axis=0)
    probs = np.empty_like(probs_p)
    probs[perm] = probs_p
    total = np.float32(sum(np.float32(res.results[c]["loss"][0, 0]) for c in range(NC)))
    mean = np.float32(total / np.float32(N))
    return (probs, mean, total), res


def kernel(**inputs):
    out, _ = _run(inputs, trace=False)
    return out


if __name__ == "__main__":
    import reference
    inputs = reference.setup_inputs()
    out, res = _run({k: np.asarray(v) for k, v in inputs.items()}, trace=False)
    print("probs", out[0].shape, out[0].dtype, "mean", out[1], "total", out[2])
